# revision 11
# baseline (speedup 1.0000x reference)
"""Trainium2 Bass kernel for MoE top-2 routing (softmax + l_aux + combine weights).

Problem: logits/mask1/mask2 [8192, 64], locations1/2 one-hot [8192, 256].
Outputs: l_aux scalar and combine_weights [8192, 64, 256].

Key structural facts exploited:
  * mask1/mask2 are disjoint one-hot rows and locations are one-hot rows, so
    combine_weights has exactly 2 nonzero ELEMENTS per token out of 64*256:
    cw[s, e_k, c_k] = g_k[s]. We scatter single f32 values at computed flat
    offsets (s*E + e_k)*C + c_k into the pre-zeroed output via indirect DMA
    instead of materializing 512 MiB. The Q7 descriptor generation
    (~2.5 us per 128-index instruction) dominates; the rest of the kernel
    hides behind the 16 scatter instructions.
  * g1 = num1/(num1+num2) with num_k = exp(top-k logit) simplifies to
    g1 = sigmoid(max - secondmax) and g2 = 1 - g1; the softmax denominator
    cancels and the eps clamp never binds.
  * l_aux only needs column sums of gates (softmax) and mask1; both are
    computed as recip/ones-weighted PE matmuls accumulated in PSUM, entirely
    off the critical path; each core emits partial sums and the host
    finishes the tiny reduction while unsharding.

Sharding: tokens split 8 ways (1024 tokens per core); no cross-core
communication needed on device. Per-core token s = 8*p + j lives on
partition p, column group j (contiguous DRAM rows per partition => large
DMA descriptors on loads).
"""

import numpy as np

import concourse.bass as bass
import concourse.bacc as bacc
import concourse.mybir as mybir
from concourse.tile import TileContext
from concourse import bass_utils

S, E, C = 8192, 64, 256
N_CORES = 8
S_LOC = S // N_CORES          # 1024 tokens per core
P = 128                       # partitions
J = S_LOC // P                # 8 tokens per partition
F32 = mybir.dt.float32
I32 = mybir.dt.int32
AX = mybir.AxisListType.X
MUL = mybir.AluOpType.mult
ADD = mybir.AluOpType.add
SUB = mybir.AluOpType.subtract
EXP = mybir.ActivationFunctionType.Exp
SIG = mybir.ActivationFunctionType.Sigmoid

_CACHE = {}


def _build():
    nc = bacc.Bacc("TRN2", target_bir_lowering=False)

    logits = nc.declare_dram_parameter("logits", [S_LOC, E], F32, isOutput=False)
    m1d = nc.declare_dram_parameter("mask1", [S_LOC, E], F32, isOutput=False)
    m2d = nc.declare_dram_parameter("mask2", [S_LOC, E], F32, isOutput=False)
    l1d = nc.declare_dram_parameter("loc1", [S_LOC, C], F32, isOutput=False)
    l2d = nc.declare_dram_parameter("loc2", [S_LOC, C], F32, isOutput=False)
    cw = nc.declare_dram_parameter("cw", [S_LOC * E * C, 1], F32, isOutput=True)
    partials = nc.declare_dram_parameter("partials", [1, 2 * E], F32, isOutput=True)

    with TileContext(nc) as tc:
        with (
            tc.tile_pool(name="sbuf", bufs=1) as pool,
            tc.tile_pool(name="psum", bufs=1, space="PSUM") as psum_pool,
        ):
            def v3(tile, inner):  # [P, J*inner] -> [P, J, inner]
                return tile[:].rearrange("p (j i) -> p j i", j=J)

            # ---- input loads (HWDGE, contiguous rows per partition) ----
            m1 = pool.tile([P, J * E], F32)
            nc.sync.dma_start(m1[:], m1d[:].rearrange("(p j) e -> p (j e)", p=P))
            lt = pool.tile([P, J * E], F32)
            nc.sync.dma_start(lt[:], logits[:].rearrange("(p j) e -> p (j e)", p=P))
            l1 = pool.tile([P, J * C], F32)
            nc.sync.dma_start(l1[:], l1d[:].rearrange("(p j) c -> p (j c)", p=P))
            m2 = pool.tile([P, J * E], F32)
            nc.sync.dma_start(m2[:], m2d[:].rearrange("(p j) e -> p (j e)", p=P))
            l2 = pool.tile([P, J * C], F32)
            nc.sync.dma_start(l2[:], l2d[:].rearrange("(p j) c -> p (j c)", p=P))

            # iota tables (no DMA): rowvals[p, j*E+e] = (8p+j)*E + e ; cvals[c] = c
            rvi = pool.tile([P, J * E], I32)
            nc.gpsimd.iota(rvi[:], pattern=[[E, J], [1, E]], base=0,
                           channel_multiplier=J * E)
            rv = pool.tile([P, J * E], F32)
            nc.vector.tensor_copy(rv[:], rvi[:])
            ic_dram = nc.inline_tensor(
                np.broadcast_to(np.arange(C, dtype=np.float32), (P, C)).copy(), "cvals")
            ic = pool.tile([P, C], F32)
            nc.sync.dma_start(ic[:], ic_dram[:])

            # ---- term-1 critical chain: row index, c index, flat offset ----
            def term_chain(k, mk, lk):
                q = pool.tile([P, J * E], F32, tag=f"q{k}")
                nc.vector.tensor_tensor(v3(q, E), v3(mk, E), v3(rv, E), op=MUL)
                rf = pool.tile([P, J], F32, tag=f"rf{k}")
                nc.vector.reduce_sum(rf[:], v3(q, E), axis=AX)
                cx = pool.tile([P, J], F32, tag=f"cx{k}")
                qc = pool.tile([P, J * C], F32, tag=f"qc{k}")
                icb = bass.AP(ic[:].tensor, ic[:].offset,
                              [ic[:].ap[0], [0, J], [1, C]])
                nc.vector.tensor_tensor(v3(qc, C), v3(lk, C), icb, op=MUL)
                nc.vector.reduce_sum(cx[:], v3(qc, C), axis=AX)
                off_f = pool.tile([P, J], F32, tag=f"offf{k}")
                nc.vector.tensor_scalar_mul(off_f[:], rf[:], float(C))
                nc.vector.tensor_tensor(off_f[:], off_f[:], cx[:], op=ADD)
                off_i = pool.tile([P, J], I32, tag=f"offi{k}")
                nc.vector.tensor_copy(off_i[:], off_f[:])
                return off_i

            off1 = term_chain(0, m1, l1)

            # ---- gate values: g1 = sigmoid(max - secondmax), g2 = 1 - g1 ----
            rmax = pool.tile([P, J], F32)
            nc.vector.reduce_max(rmax[:], v3(lt, E), axis=AX)
            p2 = pool.tile([P, J * E], F32)
            nc.vector.tensor_tensor(v3(p2, E), v3(m2, E), v3(lt, E), op=MUL)
            sm2 = pool.tile([P, J], F32)
            nc.vector.reduce_sum(sm2[:], v3(p2, E), axis=AX)
            dlt = pool.tile([P, J], F32)
            nc.vector.tensor_tensor(dlt[:], rmax[:], sm2[:], op=SUB)
            g1 = pool.tile([P, J], F32)
            nc.scalar.activation(g1[:], dlt[:], SIG)
            g2 = pool.tile([P, J], F32)
            nc.vector.tensor_scalar(g2[:], g1[:], -1.0, 1.0, op0=MUL, op1=ADD)

            # ---- term-1 scatters (window opens; term-2 chain overlaps it) ----
            def scatters(off_i, gk):
                for j in range(J):
                    nc.gpsimd.indirect_dma_start(
                        out=cw[:],
                        out_offset=bass.IndirectOffsetOnAxis(ap=off_i[:, j:j + 1], axis=0),
                        in_=gk[:, j:j + 1],
                        in_offset=None,
                    )

            scatters(off1, g1)
            off2 = term_chain(1, m2, l2)
            scatters(off2, g2)

            # ---- l_aux partials (overlap the scatter window) ----
            et = pool.tile([P, J * E], F32)
            sume = pool.tile([P, J], F32)
            for j in range(J):
                nc.scalar.activation(et[:, j * E:(j + 1) * E],
                                     lt[:, j * E:(j + 1) * E], EXP,
                                     accum_out=sume[:, j:j + 1])
            rcp = pool.tile([P, J], F32)
            nc.vector.reciprocal(rcp[:], sume[:])
            ones = pool.tile([P, 1], F32)
            nc.vector.memset(ones[:], 1.0)
            me_ps = psum_pool.tile([1, E], F32, space="PSUM")
            for j in range(J):
                nc.tensor.matmul(me_ps[:], lhsT=rcp[:, j:j + 1],
                                 rhs=et[:, j * E:(j + 1) * E],
                                 start=(j == 0), stop=(j == J - 1))
            ce_ps = psum_pool.tile([1, E], F32, space="PSUM")
            for j in range(J):
                nc.tensor.matmul(ce_ps[:], lhsT=ones[:],
                                 rhs=m1[:, j * E:(j + 1) * E],
                                 start=(j == 0), stop=(j == J - 1))
            part_sb = pool.tile([1, 2 * E], F32)
            nc.vector.tensor_copy(part_sb[:1, :E], me_ps[:])
            nc.vector.tensor_copy(part_sb[:1, E:], ce_ps[:])
            nc.sync.dma_start(partials[:], part_sb[:])
    nc.finalize()
    return nc


def _get_nc():
    if "nc" not in _CACHE:
        _CACHE["nc"] = _build()
    return _CACHE["nc"]


def _in_maps(logits, mask1_float, mask2_float, locations1_sc, locations2_sc):
    maps = []
    for c in range(N_CORES):
        sl = slice(c * S_LOC, (c + 1) * S_LOC)
        maps.append({
            "logits": np.ascontiguousarray(logits[sl]),
            "mask1": np.ascontiguousarray(mask1_float[sl]),
            "mask2": np.ascontiguousarray(mask2_float[sl]),
            "loc1": np.ascontiguousarray(locations1_sc[sl]),
            "loc2": np.ascontiguousarray(locations2_sc[sl]),
        })
    return maps


def _install_ntff_shim():
    """The agent image's antenv lacks axon_hooks; provide it so trace=True
    can capture NTFF profiles via the libaxon ctypes path."""
    import sys
    import types

    if "antenv.axon_hooks" in sys.modules:
        return
    try:
        import antenv
        from trn_agent_boot.trn_boot import _ntff_profile_via_ctypes

        mod = types.ModuleType("antenv.axon_hooks")
        hook = _ntff_profile_via_ctypes("/opt/axon/libaxon_pjrt.so")
        mod._hook = hook
        mod.set_axon_ntff_profile_hook = lambda h: setattr(mod, "_hook", h)
        mod.get_axon_ntff_profile_hook = lambda: mod._hook
        sys.modules["antenv.axon_hooks"] = mod
        antenv.axon_hooks = mod
    except Exception:
        pass


def _run(inputs, trace=False, **kwargs):
    if trace:
        _install_ntff_shim()
    nc = _get_nc()
    maps = _in_maps(**{k: np.asarray(v) for k, v in inputs.items()})
    return bass_utils.run_bass_kernel_spmd(
        nc, maps, core_ids=list(range(N_CORES)), trace=trace, **kwargs
    )


def _assemble(results):
    cw = np.concatenate(
        [results[c]["cw"].reshape(S_LOC, E, C) for c in range(N_CORES)], axis=0
    )
    me_sum = np.zeros(E, np.float64)
    ce_sum = np.zeros(E, np.float64)
    for c in range(N_CORES):
        part = results[c]["partials"].reshape(2 * E)
        me_sum += part[:E]
        ce_sum += part[E:]
    l_aux = np.float32(E * np.sum(me_sum * ce_sum) / (S * S))
    return l_aux, cw


def kernel(**inputs):
    res = _run(inputs)
    return _assemble(res.results)


# revision 12
# speedup vs baseline: 1.0940x; 1.0940x over previous
"""Trainium2 Bass kernel for MoE top-2 routing (softmax + l_aux + combine weights).

Problem: logits/mask1/mask2 [8192, 64], locations1/2 one-hot [8192, 256].
Outputs: l_aux scalar and combine_weights [8192, 64, 256].

Key structural facts exploited:
  * mask1/mask2 are disjoint one-hot rows and locations are one-hot rows, so
    combine_weights has exactly 2 nonzero ELEMENTS per token out of 64*256:
    cw[s, e_k, c_k] = g_k[s]. We scatter single f32 values at computed flat
    offsets (s*E + e_k)*C + c_k into the pre-zeroed output via indirect DMA
    instead of materializing 512 MiB. The Q7 descriptor generation
    (~2.5 us per 128-index instruction) dominates; the rest of the kernel
    hides behind the 16 scatter instructions.
  * g1 = num1/(num1+num2) with num_k = exp(top-k logit) simplifies to
    g1 = sigmoid(max - secondmax) and g2 = 1 - g1; the softmax denominator
    cancels and the eps clamp never binds.
  * l_aux only needs column sums of gates (softmax) and mask1; both are
    computed as recip/ones-weighted PE matmuls accumulated in PSUM, entirely
    off the critical path; each core emits partial sums and the host
    finishes the tiny reduction while unsharding.

Sharding: tokens split 8 ways (1024 tokens per core); no cross-core
communication needed on device. Per-core token s = 8*p + j lives on
partition p, column group j (contiguous DRAM rows per partition => large
DMA descriptors on loads).
"""

import numpy as np

import concourse.bass as bass
import concourse.bacc as bacc
import concourse.mybir as mybir
from concourse.tile import TileContext
from concourse import bass_utils

S, E, C = 8192, 64, 256
N_CORES = 8
S_LOC = S // N_CORES          # 1024 tokens per core
P = 128                       # partitions
J = S_LOC // P                # 8 tokens per partition
F32 = mybir.dt.float32
I32 = mybir.dt.int32
BF16 = mybir.dt.bfloat16
AX = mybir.AxisListType.X
MUL = mybir.AluOpType.mult
ADD = mybir.AluOpType.add
SUB = mybir.AluOpType.subtract
EXP = mybir.ActivationFunctionType.Exp
SIG = mybir.ActivationFunctionType.Sigmoid

import ml_dtypes
_BF16_NP = ml_dtypes.bfloat16

_CACHE = {}


def _build():
    nc = bacc.Bacc("TRN2", target_bir_lowering=False)

    logits = nc.declare_dram_parameter("logits", [S_LOC, E], F32, isOutput=False)
    m1d = nc.declare_dram_parameter("mask1", [S_LOC, E], F32, isOutput=False)
    m2d = nc.declare_dram_parameter("mask2", [S_LOC, E], F32, isOutput=False)
    l1d = nc.declare_dram_parameter("loc1", [S_LOC, C], BF16, isOutput=False)
    l2d = nc.declare_dram_parameter("loc2", [S_LOC, C], BF16, isOutput=False)
    cw = nc.declare_dram_parameter("cw", [S_LOC * E * C, 1], F32, isOutput=True)
    partials = nc.declare_dram_parameter("partials", [1, 2 * E], F32, isOutput=True)

    with TileContext(nc) as tc:
        with (
            tc.tile_pool(name="sbuf", bufs=1) as pool,
            tc.tile_pool(name="psum", bufs=1, space="PSUM") as psum_pool,
        ):
            def v3(tile, inner):  # [P, J*inner] -> [P, J, inner]
                return tile[:].rearrange("p (j i) -> p j i", j=J)

            # ---- input loads (HWDGE, contiguous rows per partition) ----
            m1 = pool.tile([P, J * E], F32)
            nc.sync.dma_start(m1[:], m1d[:].rearrange("(p j) e -> p (j e)", p=P))
            lt = pool.tile([P, J * E], F32)
            nc.sync.dma_start(lt[:], logits[:].rearrange("(p j) e -> p (j e)", p=P))
            m2 = pool.tile([P, J * E], F32)
            nc.sync.dma_start(m2[:], m2d[:].rearrange("(p j) e -> p (j e)", p=P))
            l1 = pool.tile([P, J * C], BF16)
            nc.sync.dma_start(l1[:], l1d[:].rearrange("(p j) c -> p (j c)", p=P))
            l2 = pool.tile([P, J * C], BF16)
            nc.sync.dma_start(l2[:], l2d[:].rearrange("(p j) c -> p (j c)", p=P))

            # iota tables (no DMA): rowvals[p, j*E+e] = (8p+j)*E + e ; cvals[c] = c
            rvi = pool.tile([P, J * E], I32)
            nc.gpsimd.iota(rvi[:], pattern=[[E, J], [1, E]], base=0,
                           channel_multiplier=J * E)
            rv = pool.tile([P, J * E], F32)
            nc.vector.tensor_copy(rv[:], rvi[:])
            import ml_dtypes
            ic_dram = nc.inline_tensor(
                np.broadcast_to(np.arange(C).astype(ml_dtypes.bfloat16), (P, C)).copy(),
                "cvals")
            ic = pool.tile([P, C], BF16)
            nc.sync.dma_start(ic[:], ic_dram[:])

            # ---- term-1 critical chain: row index, c index, flat offset ----
            def term_chain(k, mk, lk):
                q = pool.tile([P, J * E], F32, tag=f"q{k}")
                nc.vector.tensor_tensor(v3(q, E), v3(mk, E), v3(rv, E), op=MUL)
                rf = pool.tile([P, J], F32, tag=f"rf{k}")
                nc.vector.reduce_sum(rf[:], v3(q, E), axis=AX)
                cx = pool.tile([P, J], F32, tag=f"cx{k}")
                qc = pool.tile([P, J * C], BF16, tag=f"qc{k}")
                icb = bass.AP(ic[:].tensor, ic[:].offset,
                              [ic[:].ap[0], [0, J], [1, C]])
                nc.vector.tensor_tensor(v3(qc, C), v3(lk, C), icb, op=MUL)
                nc.vector.reduce_sum(cx[:], v3(qc, C), axis=AX)
                off_f = pool.tile([P, J], F32, tag=f"offf{k}")
                nc.vector.tensor_scalar_mul(off_f[:], rf[:], float(C))
                nc.vector.tensor_tensor(off_f[:], off_f[:], cx[:], op=ADD)
                off_i = pool.tile([P, J], I32, tag=f"offi{k}")
                nc.vector.tensor_copy(off_i[:], off_f[:])
                return off_i

            off1 = term_chain(0, m1, l1)

            # ---- gate values: g1 = sigmoid(max - secondmax), g2 = 1 - g1 ----
            rmax = pool.tile([P, J], F32)
            nc.vector.reduce_max(rmax[:], v3(lt, E), axis=AX)
            p2 = pool.tile([P, J * E], F32)
            nc.vector.tensor_tensor(v3(p2, E), v3(m2, E), v3(lt, E), op=MUL)
            sm2 = pool.tile([P, J], F32)
            nc.vector.reduce_sum(sm2[:], v3(p2, E), axis=AX)
            dlt = pool.tile([P, J], F32)
            nc.vector.tensor_tensor(dlt[:], rmax[:], sm2[:], op=SUB)
            g1 = pool.tile([P, J], F32)
            nc.scalar.activation(g1[:], dlt[:], SIG)
            g2 = pool.tile([P, J], F32)
            nc.vector.tensor_scalar(g2[:], g1[:], -1.0, 1.0, op0=MUL, op1=ADD)

            # ---- term-1 scatters (window opens; term-2 chain overlaps it) ----
            def scatters(off_i, gk):
                for j in range(J):
                    nc.gpsimd.indirect_dma_start(
                        out=cw[:],
                        out_offset=bass.IndirectOffsetOnAxis(ap=off_i[:, j:j + 1], axis=0),
                        in_=gk[:, j:j + 1],
                        in_offset=None,
                    )

            scatters(off1, g1)
            off2 = term_chain(1, m2, l2)
            scatters(off2, g2)

            # ---- l_aux partials (overlap the scatter window) ----
            et = pool.tile([P, J * E], F32)
            sume = pool.tile([P, J], F32)
            for j in range(J):
                nc.scalar.activation(et[:, j * E:(j + 1) * E],
                                     lt[:, j * E:(j + 1) * E], EXP,
                                     accum_out=sume[:, j:j + 1])
            rcp = pool.tile([P, J], F32)
            nc.vector.reciprocal(rcp[:], sume[:])
            ones = pool.tile([P, 1], F32)
            nc.vector.memset(ones[:], 1.0)
            me_ps = psum_pool.tile([1, E], F32, space="PSUM")
            for j in range(J):
                nc.tensor.matmul(me_ps[:], lhsT=rcp[:, j:j + 1],
                                 rhs=et[:, j * E:(j + 1) * E],
                                 start=(j == 0), stop=(j == J - 1))
            ce_ps = psum_pool.tile([1, E], F32, space="PSUM")
            for j in range(J):
                nc.tensor.matmul(ce_ps[:], lhsT=ones[:],
                                 rhs=m1[:, j * E:(j + 1) * E],
                                 start=(j == 0), stop=(j == J - 1))
            part_sb = pool.tile([1, 2 * E], F32)
            nc.vector.tensor_copy(part_sb[:1, :E], me_ps[:])
            nc.vector.tensor_copy(part_sb[:1, E:], ce_ps[:])
            nc.sync.dma_start(partials[:], part_sb[:])
    nc.finalize()
    return nc


def _get_nc():
    if "nc" not in _CACHE:
        _CACHE["nc"] = _build()
    return _CACHE["nc"]


def _in_maps(logits, mask1_float, mask2_float, locations1_sc, locations2_sc):
    maps = []
    for c in range(N_CORES):
        sl = slice(c * S_LOC, (c + 1) * S_LOC)
        maps.append({
            "logits": np.ascontiguousarray(logits[sl]),
            "mask1": np.ascontiguousarray(mask1_float[sl]),
            "mask2": np.ascontiguousarray(mask2_float[sl]),
            "loc1": np.ascontiguousarray(locations1_sc[sl]).astype(_BF16_NP),
            "loc2": np.ascontiguousarray(locations2_sc[sl]).astype(_BF16_NP),
        })
    return maps


def _install_ntff_shim():
    """The agent image's antenv lacks axon_hooks; provide it so trace=True
    can capture NTFF profiles via the libaxon ctypes path."""
    import sys
    import types

    if "antenv.axon_hooks" in sys.modules:
        return
    try:
        import antenv
        from trn_agent_boot.trn_boot import _ntff_profile_via_ctypes

        mod = types.ModuleType("antenv.axon_hooks")
        hook = _ntff_profile_via_ctypes("/opt/axon/libaxon_pjrt.so")
        mod._hook = hook
        mod.set_axon_ntff_profile_hook = lambda h: setattr(mod, "_hook", h)
        mod.get_axon_ntff_profile_hook = lambda: mod._hook
        sys.modules["antenv.axon_hooks"] = mod
        antenv.axon_hooks = mod
    except Exception:
        pass


def _run(inputs, trace=False, **kwargs):
    if trace:
        _install_ntff_shim()
    nc = _get_nc()
    maps = _in_maps(**{k: np.asarray(v) for k, v in inputs.items()})
    return bass_utils.run_bass_kernel_spmd(
        nc, maps, core_ids=list(range(N_CORES)), trace=trace, **kwargs
    )


def _assemble(results):
    cw = np.concatenate(
        [results[c]["cw"].reshape(S_LOC, E, C) for c in range(N_CORES)], axis=0
    )
    me_sum = np.zeros(E, np.float64)
    ce_sum = np.zeros(E, np.float64)
    for c in range(N_CORES):
        part = results[c]["partials"].reshape(2 * E)
        me_sum += part[:E]
        ce_sum += part[E:]
    l_aux = np.float32(E * np.sum(me_sum * ce_sum) / (S * S))
    return l_aux, cw


def kernel(**inputs):
    res = _run(inputs)
    return _assemble(res.results)


# revision 14
# speedup vs baseline: 1.0985x; 1.0041x over previous
"""Trainium2 Bass kernel for MoE top-2 routing (softmax + l_aux + combine weights).

Problem: logits/mask1/mask2 [8192, 64], locations1/2 one-hot [8192, 256].
Outputs: l_aux scalar and combine_weights [8192, 64, 256].

Key structural facts exploited:
  * mask1/mask2 are disjoint one-hot rows and locations are one-hot rows, so
    combine_weights has exactly 2 nonzero ELEMENTS per token out of 64*256:
    cw[s, e_k, c_k] = g_k[s]. We scatter single f32 values at computed flat
    offsets (s*E + e_k)*C + c_k into the pre-zeroed output via indirect DMA
    instead of materializing 512 MiB. The Q7 descriptor generation
    (~2.5 us per 128-index instruction) dominates; the rest of the kernel
    hides behind the 16 scatter instructions.
  * g1 = num1/(num1+num2) with num_k = exp(top-k logit) simplifies to
    g1 = sigmoid(max - secondmax) and g2 = 1 - g1; the softmax denominator
    cancels and the eps clamp never binds.
  * l_aux only needs column sums of gates (softmax) and mask1; both are
    computed as recip/ones-weighted PE matmuls accumulated in PSUM, entirely
    off the critical path; each core emits partial sums and the host
    finishes the tiny reduction while unsharding.

Sharding: tokens split 8 ways (1024 tokens per core); no cross-core
communication needed on device. Per-core token s = 8*p + j lives on
partition p, column group j (contiguous DRAM rows per partition => large
DMA descriptors on loads).
"""

import numpy as np

import concourse.bass as bass
import concourse.bacc as bacc
import concourse.mybir as mybir
from concourse.tile import TileContext
from concourse import bass_utils

S, E, C = 8192, 64, 256
N_CORES = 8
S_LOC = S // N_CORES          # 1024 tokens per core
P = 128                       # partitions
J = S_LOC // P                # 8 tokens per partition
F32 = mybir.dt.float32
I32 = mybir.dt.int32
BF16 = mybir.dt.bfloat16
AX = mybir.AxisListType.X
MUL = mybir.AluOpType.mult
ADD = mybir.AluOpType.add
SUB = mybir.AluOpType.subtract
EXP = mybir.ActivationFunctionType.Exp
SIG = mybir.ActivationFunctionType.Sigmoid

import ml_dtypes
_BF16_NP = ml_dtypes.bfloat16

_CACHE = {}


def _build():
    nc = bacc.Bacc("TRN2", target_bir_lowering=False)

    logits = nc.declare_dram_parameter("logits", [S_LOC, E], F32, isOutput=False)
    m1d = nc.declare_dram_parameter("mask1", [S_LOC, E], F32, isOutput=False)
    m2d = nc.declare_dram_parameter("mask2", [S_LOC, E], F32, isOutput=False)
    l1d = nc.declare_dram_parameter("loc1", [S_LOC, C], BF16, isOutput=False)
    l2d = nc.declare_dram_parameter("loc2", [S_LOC, C], BF16, isOutput=False)
    cw = nc.declare_dram_parameter("cw", [S_LOC * E * C, 1], F32, isOutput=True)
    partials = nc.declare_dram_parameter("partials", [1, 2 * E], F32, isOutput=True)

    with TileContext(nc) as tc:
        with (
            tc.tile_pool(name="sbuf", bufs=1) as pool,
            tc.tile_pool(name="psum", bufs=1, space="PSUM") as psum_pool,
        ):
            def v3(tile, inner):  # [P, J*inner] -> [P, J, inner]
                return tile[:].rearrange("p (j i) -> p j i", j=J)

            # ---- input loads (HWDGE, contiguous rows per partition) ----
            m1 = pool.tile([P, J * E], F32)
            nc.sync.dma_start(m1[:], m1d[:].rearrange("(p j) e -> p (j e)", p=P))
            l1 = pool.tile([P, J * C], BF16)
            nc.sync.dma_start(l1[:], l1d[:].rearrange("(p j) c -> p (j c)", p=P))
            lt = pool.tile([P, J * E], F32)
            nc.sync.dma_start(lt[:], logits[:].rearrange("(p j) e -> p (j e)", p=P))
            m2 = pool.tile([P, J * E], F32)
            nc.sync.dma_start(m2[:], m2d[:].rearrange("(p j) e -> p (j e)", p=P))
            l2 = pool.tile([P, J * C], BF16)
            nc.sync.dma_start(l2[:], l2d[:].rearrange("(p j) c -> p (j c)", p=P))

            # iota tables (no DMA): rowvals[p, j*E+e] = (8p+j)*E + e ; cvals[c] = c
            rvi = pool.tile([P, J * E], I32)
            nc.gpsimd.iota(rvi[:], pattern=[[E, J], [1, E]], base=0,
                           channel_multiplier=J * E)
            rv = pool.tile([P, J * E], F32)
            nc.vector.tensor_copy(rv[:], rvi[:])
            import ml_dtypes
            ic_dram = nc.inline_tensor(
                np.broadcast_to(np.arange(C).astype(ml_dtypes.bfloat16), (P, C)).copy(),
                "cvals")
            ic = pool.tile([P, C], BF16)
            nc.sync.dma_start(ic[:], ic_dram[:])

            # ---- term-1 critical chain: row index, c index, flat offset ----
            def term_chain(k, mk, lk):
                q = pool.tile([P, J * E], F32, tag=f"q{k}")
                nc.vector.tensor_tensor(v3(q, E), v3(mk, E), v3(rv, E), op=MUL)
                rf = pool.tile([P, J], F32, tag=f"rf{k}")
                nc.vector.reduce_sum(rf[:], v3(q, E), axis=AX)
                cx = pool.tile([P, J], BF16, tag=f"cx{k}")
                qc = pool.tile([P, J * C], BF16, tag=f"qc{k}")
                icb = bass.AP(ic[:].tensor, ic[:].offset,
                              [ic[:].ap[0], [0, J], [1, C]])
                nc.vector.tensor_tensor(v3(qc, C), v3(lk, C), icb, op=MUL)
                with nc.allow_low_precision(reason="one-hot select of ints <=255; exact in bf16"):
                    nc.vector.reduce_sum(cx[:], v3(qc, C), axis=AX)
                off_f = pool.tile([P, J], F32, tag=f"offf{k}")
                nc.vector.tensor_scalar_mul(off_f[:], rf[:], float(C))
                nc.vector.tensor_tensor(off_f[:], off_f[:], cx[:], op=ADD)
                off_i = pool.tile([P, J], I32, tag=f"offi{k}")
                nc.vector.tensor_copy(off_i[:], off_f[:])
                return off_i

            off1 = term_chain(0, m1, l1)

            # ---- gate values: g1 = sigmoid(max - secondmax), g2 = 1 - g1 ----
            rmax = pool.tile([P, J], F32)
            nc.vector.reduce_max(rmax[:], v3(lt, E), axis=AX)
            p2 = pool.tile([P, J * E], F32)
            nc.vector.tensor_tensor(v3(p2, E), v3(m2, E), v3(lt, E), op=MUL)
            sm2 = pool.tile([P, J], F32)
            nc.vector.reduce_sum(sm2[:], v3(p2, E), axis=AX)
            dltn = pool.tile([P, J], F32)
            nc.vector.tensor_tensor(dltn[:], sm2[:], rmax[:], op=SUB)
            e2 = pool.tile([P, J], F32)
            nc.scalar.activation(e2[:], dltn[:], EXP)
            den = pool.tile([P, J], F32)
            nc.vector.tensor_scalar_add(den[:], e2[:], 1.0)
            g1 = pool.tile([P, J], F32)
            nc.vector.reciprocal(g1[:], den[:])
            g2 = pool.tile([P, J], F32)
            nc.vector.tensor_tensor(g2[:], e2[:], g1[:], op=MUL)

            # ---- term-1 scatters (window opens; term-2 chain overlaps it) ----
            def scatters(off_i, gk):
                for j in range(J):
                    nc.gpsimd.indirect_dma_start(
                        out=cw[:],
                        out_offset=bass.IndirectOffsetOnAxis(ap=off_i[:, j:j + 1], axis=0),
                        in_=gk[:, j:j + 1],
                        in_offset=None,
                    )

            scatters(off1, g1)
            off2 = term_chain(1, m2, l2)
            scatters(off2, g2)

            # ---- l_aux partials (overlap the scatter window) ----
            et = pool.tile([P, J * E], F32)
            sume = pool.tile([P, J], F32)
            for j in range(J):
                nc.scalar.activation(et[:, j * E:(j + 1) * E],
                                     lt[:, j * E:(j + 1) * E], EXP,
                                     accum_out=sume[:, j:j + 1])
            rcp = pool.tile([P, J], F32)
            nc.vector.reciprocal(rcp[:], sume[:])
            ones = pool.tile([P, 1], F32)
            nc.vector.memset(ones[:], 1.0)
            me_ps = psum_pool.tile([1, E], F32, space="PSUM")
            for j in range(J):
                nc.tensor.matmul(me_ps[:], lhsT=rcp[:, j:j + 1],
                                 rhs=et[:, j * E:(j + 1) * E],
                                 start=(j == 0), stop=(j == J - 1))
            ce_ps = psum_pool.tile([1, E], F32, space="PSUM")
            for j in range(J):
                nc.tensor.matmul(ce_ps[:], lhsT=ones[:],
                                 rhs=m1[:, j * E:(j + 1) * E],
                                 start=(j == 0), stop=(j == J - 1))
            part_sb = pool.tile([1, 2 * E], F32)
            nc.vector.tensor_copy(part_sb[:1, :E], me_ps[:])
            nc.vector.tensor_copy(part_sb[:1, E:], ce_ps[:])
            nc.sync.dma_start(partials[:], part_sb[:])
    nc.finalize()
    return nc


def _get_nc():
    if "nc" not in _CACHE:
        _CACHE["nc"] = _build()
    return _CACHE["nc"]


def _in_maps(logits, mask1_float, mask2_float, locations1_sc, locations2_sc):
    maps = []
    for c in range(N_CORES):
        sl = slice(c * S_LOC, (c + 1) * S_LOC)
        maps.append({
            "logits": np.ascontiguousarray(logits[sl]),
            "mask1": np.ascontiguousarray(mask1_float[sl]),
            "mask2": np.ascontiguousarray(mask2_float[sl]),
            "loc1": np.ascontiguousarray(locations1_sc[sl]).astype(_BF16_NP),
            "loc2": np.ascontiguousarray(locations2_sc[sl]).astype(_BF16_NP),
        })
    return maps


def _install_ntff_shim():
    """The agent image's antenv lacks axon_hooks; provide it so trace=True
    can capture NTFF profiles via the libaxon ctypes path."""
    import sys
    import types

    if "antenv.axon_hooks" in sys.modules:
        return
    try:
        import antenv
        from trn_agent_boot.trn_boot import _ntff_profile_via_ctypes

        mod = types.ModuleType("antenv.axon_hooks")
        hook = _ntff_profile_via_ctypes("/opt/axon/libaxon_pjrt.so")
        mod._hook = hook
        mod.set_axon_ntff_profile_hook = lambda h: setattr(mod, "_hook", h)
        mod.get_axon_ntff_profile_hook = lambda: mod._hook
        sys.modules["antenv.axon_hooks"] = mod
        antenv.axon_hooks = mod
    except Exception:
        pass


def _run(inputs, trace=False, **kwargs):
    if trace:
        _install_ntff_shim()
    nc = _get_nc()
    maps = _in_maps(**{k: np.asarray(v) for k, v in inputs.items()})
    return bass_utils.run_bass_kernel_spmd(
        nc, maps, core_ids=list(range(N_CORES)), trace=trace, **kwargs
    )


def _assemble(results):
    cw = np.concatenate(
        [results[c]["cw"].reshape(S_LOC, E, C) for c in range(N_CORES)], axis=0
    )
    me_sum = np.zeros(E, np.float64)
    ce_sum = np.zeros(E, np.float64)
    for c in range(N_CORES):
        part = results[c]["partials"].reshape(2 * E)
        me_sum += part[:E]
        ce_sum += part[E:]
    l_aux = np.float32(E * np.sum(me_sum * ce_sum) / (S * S))
    return l_aux, cw


def kernel(**inputs):
    res = _run(inputs)
    return _assemble(res.results)


# revision 15
# speedup vs baseline: 1.1704x; 1.0654x over previous
"""Trainium2 Bass kernel for MoE top-2 routing (softmax + l_aux + combine weights).

Problem: logits/mask1/mask2 [8192, 64], locations1/2 one-hot [8192, 256].
Outputs: l_aux scalar and combine_weights [8192, 64, 256].

Key structural facts exploited:
  * mask1/mask2 are disjoint one-hot rows and locations are one-hot rows, so
    combine_weights has exactly 2 nonzero ELEMENTS per token out of 64*256:
    cw[s, e_k, c_k] = g_k[s]. We scatter single f32 values at computed flat
    offsets (s*E + e_k)*C + c_k into the pre-zeroed output via indirect DMA
    instead of materializing 512 MiB. The Q7 descriptor generation
    (~2.5 us per 128-index instruction) dominates; the rest of the kernel
    hides behind the 16 scatter instructions.
  * g1 = num1/(num1+num2) with num_k = exp(top-k logit) simplifies to
    g1 = sigmoid(max - secondmax) and g2 = 1 - g1; the softmax denominator
    cancels and the eps clamp never binds.
  * l_aux only needs column sums of gates (softmax) and mask1; both are
    computed as recip/ones-weighted PE matmuls accumulated in PSUM, entirely
    off the critical path; each core emits partial sums and the host
    finishes the tiny reduction while unsharding.

Sharding: tokens split 8 ways (1024 tokens per core); no cross-core
communication needed on device. Per-core token s = 8*p + j lives on
partition p, column group j (contiguous DRAM rows per partition => large
DMA descriptors on loads).
"""

import numpy as np

import concourse.bass as bass
import concourse.bacc as bacc
import concourse.mybir as mybir
from concourse.tile import TileContext, add_dep_helper
from concourse import bass_utils

S, E, C = 8192, 64, 256
N_CORES = 8
S_LOC = S // N_CORES          # 1024 tokens per core
P = 128                       # partitions
J = S_LOC // P                # 8 tokens per partition
F32 = mybir.dt.float32
I32 = mybir.dt.int32
BF16 = mybir.dt.bfloat16
AX = mybir.AxisListType.X
MUL = mybir.AluOpType.mult
ADD = mybir.AluOpType.add
SUB = mybir.AluOpType.subtract
EXP = mybir.ActivationFunctionType.Exp
SIG = mybir.ActivationFunctionType.Sigmoid

import ml_dtypes
_BF16_NP = ml_dtypes.bfloat16

_CACHE = {}


def _last_inst(nc):
    return nc.inst_map[list(nc.inst_map)[-1]]


def _build():
    nc = bacc.Bacc("TRN2", target_bir_lowering=False)

    logits = nc.declare_dram_parameter("logits", [S_LOC, E], F32, isOutput=False)
    m1d = nc.declare_dram_parameter("mask1", [S_LOC, E], F32, isOutput=False)
    m2d = nc.declare_dram_parameter("mask2", [S_LOC, E], F32, isOutput=False)
    l1d = nc.declare_dram_parameter("loc1", [S_LOC, C], BF16, isOutput=False)
    l2d = nc.declare_dram_parameter("loc2", [S_LOC, C], BF16, isOutput=False)
    cw = nc.declare_dram_parameter("cw", [S_LOC * E * C, 1], F32, isOutput=True)
    partials = nc.declare_dram_parameter("partials", [1, 2 * E], F32, isOutput=True)

    with TileContext(nc) as tc:
        with (
            tc.tile_pool(name="sbuf", bufs=1) as pool,
            tc.tile_pool(name="psum", bufs=1, space="PSUM") as psum_pool,
        ):
            def v3(tile, inner):  # [P, J*inner] -> [P, J, inner]
                return tile[:].rearrange("p (j i) -> p j i", j=J)

            # ---- input loads (HWDGE, contiguous rows per partition) ----
            m1 = pool.tile([P, J * E], F32)
            nc.sync.dma_start(m1[:], m1d[:].rearrange("(p j) e -> p (j e)", p=P))
            l1 = pool.tile([P, J * C], BF16)
            nc.scalar.dma_start(l1[:], l1d[:].rearrange("(p j) c -> p (j c)", p=P))
            lt = pool.tile([P, J * E], F32)
            nc.sync.dma_start(lt[:], logits[:].rearrange("(p j) e -> p (j e)", p=P))
            m2 = pool.tile([P, J * E], F32)
            nc.sync.dma_start(m2[:], m2d[:].rearrange("(p j) e -> p (j e)", p=P))
            l2 = pool.tile([P, J * C], BF16)
            nc.scalar.dma_start(l2[:], l2d[:].rearrange("(p j) c -> p (j c)", p=P))

            # iota tables (no DMA): rowvals[p, j*E+e] = (8p+j)*E + e ; cvals[c] = c
            rvi = pool.tile([P, J * E], I32)
            nc.gpsimd.iota(rvi[:], pattern=[[E * C, J], [C, E]], base=0,
                           channel_multiplier=J * E * C)
            rv = pool.tile([P, J * E], F32)
            nc.vector.tensor_copy(rv[:], rvi[:])
            import ml_dtypes
            ic_dram = nc.inline_tensor(
                np.broadcast_to(np.arange(C).astype(ml_dtypes.bfloat16), (P, C)).copy(),
                "cvals")
            ic = pool.tile([P, C], BF16)
            nc.scalar.dma_start(ic[:], ic_dram[:])

            # ---- term-1 critical chain: row index, c index, flat offset ----
            def term_chain(k, mk, lk, after=None, red_after=None):
                q = pool.tile([P, J * E], F32, tag=f"q{k}")
                nc.vector.tensor_tensor(v3(q, E), v3(mk, E), v3(rv, E), op=MUL)
                if after is not None:
                    add_dep_helper(_last_inst(nc), after, sync=False,
                                   reason="term2 after term1 offsets")
                rf = pool.tile([P, J], F32, tag=f"rf{k}")
                nc.vector.reduce_sum(rf[:], v3(q, E), axis=AX)
                cx = pool.tile([P, J], BF16, tag=f"cx{k}")
                qc = pool.tile([P, J * C], BF16, tag=f"qc{k}")
                icb = bass.AP(ic[:].tensor, ic[:].offset,
                              [ic[:].ap[0], [0, J], [1, C]])
                nc.vector.tensor_tensor(v3(qc, C), v3(lk, C), icb, op=MUL)
                if after is not None:
                    add_dep_helper(_last_inst(nc), after, sync=False,
                                   reason="term2 after term1 offsets")
                with nc.allow_low_precision(reason="one-hot select of ints <=255; exact in bf16"):
                    red = nc.vector.reduce_sum(cx[:], v3(qc, C), axis=AX)
                red_inst = _last_inst(nc)
                if red_after is not None:
                    add_dep_helper(red_inst, red_after, sync=False,
                                   reason="big reduce after tiny gate ops")
                off_f = pool.tile([P, J], F32, tag=f"offf{k}")
                nc.vector.tensor_tensor(off_f[:], rf[:], cx[:], op=ADD)
                off_i = pool.tile([P, J], I32, tag=f"offi{k}")
                nc.vector.tensor_copy(off_i[:], off_f[:])
                return off_i, _last_inst(nc)

            off1_pending = term_chain(0, m1, l1)

            # ---- gate values: g1 = sigmoid(max - secondmax), g2 = 1 - g1 ----
            rmax = pool.tile([P, J], F32)
            nc.vector.reduce_max(rmax[:], v3(lt, E), axis=AX)
            p2 = pool.tile([P, J * E], F32)
            nc.vector.tensor_tensor(v3(p2, E), v3(m2, E), v3(lt, E), op=MUL)
            sm2 = pool.tile([P, J], F32)
            nc.vector.reduce_sum(sm2[:], v3(p2, E), axis=AX)
            dltn = pool.tile([P, J], F32)
            nc.vector.tensor_tensor(dltn[:], sm2[:], rmax[:], op=SUB)
            e2 = pool.tile([P, J], F32)
            nc.scalar.activation(e2[:], dltn[:], EXP)
            den = pool.tile([P, J], F32)
            nc.vector.tensor_scalar_add(den[:], e2[:], 1.0)
            g1 = pool.tile([P, J], F32)
            nc.vector.reciprocal(g1[:], den[:])
            g2 = pool.tile([P, J], F32)
            nc.vector.tensor_tensor(g2[:], e2[:], g1[:], op=MUL)
            g2_inst = _last_inst(nc)

            # ---- term-1 scatters (window opens; term-2 chain overlaps it) ----
            def scatters(off_i, gk):
                for j in range(J):
                    nc.gpsimd.indirect_dma_start(
                        out=cw[:],
                        out_offset=bass.IndirectOffsetOnAxis(ap=off_i[:, j:j + 1], axis=0),
                        in_=gk[:, j:j + 1],
                        in_offset=None,
                    )

            off1, off1_cast = off1_pending
            scatters(off1, g1)
            off2, _ = term_chain(1, m2, l2, after=off1_cast, red_after=off1_cast)
            scatters(off2, g2)

            # ---- l_aux partials (overlap the scatter window) ----
            et = pool.tile([P, J * E], F32)
            sume = pool.tile([P, J], F32)
            for j in range(J):
                nc.scalar.activation(et[:, j * E:(j + 1) * E],
                                     lt[:, j * E:(j + 1) * E], EXP,
                                     accum_out=sume[:, j:j + 1])
            rcp = pool.tile([P, J], F32)
            nc.vector.reciprocal(rcp[:], sume[:])
            ones = pool.tile([P, 1], F32)
            nc.vector.memset(ones[:], 1.0)
            me_ps = psum_pool.tile([1, E], F32, space="PSUM")
            for j in range(J):
                nc.tensor.matmul(me_ps[:], lhsT=rcp[:, j:j + 1],
                                 rhs=et[:, j * E:(j + 1) * E],
                                 start=(j == 0), stop=(j == J - 1))
            ce_ps = psum_pool.tile([1, E], F32, space="PSUM")
            for j in range(J):
                nc.tensor.matmul(ce_ps[:], lhsT=ones[:],
                                 rhs=m1[:, j * E:(j + 1) * E],
                                 start=(j == 0), stop=(j == J - 1))
            part_sb = pool.tile([1, 2 * E], F32)
            nc.vector.tensor_copy(part_sb[:1, :E], me_ps[:])
            nc.vector.tensor_copy(part_sb[:1, E:], ce_ps[:])
            nc.sync.dma_start(partials[:], part_sb[:])
    nc.finalize()
    return nc


def _get_nc():
    if "nc" not in _CACHE:
        _CACHE["nc"] = _build()
    return _CACHE["nc"]


def _in_maps(logits, mask1_float, mask2_float, locations1_sc, locations2_sc):
    maps = []
    for c in range(N_CORES):
        sl = slice(c * S_LOC, (c + 1) * S_LOC)
        maps.append({
            "logits": np.ascontiguousarray(logits[sl]),
            "mask1": np.ascontiguousarray(mask1_float[sl]),
            "mask2": np.ascontiguousarray(mask2_float[sl]),
            "loc1": np.ascontiguousarray(locations1_sc[sl]).astype(_BF16_NP),
            "loc2": np.ascontiguousarray(locations2_sc[sl]).astype(_BF16_NP),
        })
    return maps


def _install_ntff_shim():
    """The agent image's antenv lacks axon_hooks; provide it so trace=True
    can capture NTFF profiles via the libaxon ctypes path."""
    import sys
    import types

    if "antenv.axon_hooks" in sys.modules:
        return
    try:
        import antenv
        from trn_agent_boot.trn_boot import _ntff_profile_via_ctypes

        mod = types.ModuleType("antenv.axon_hooks")
        hook = _ntff_profile_via_ctypes("/opt/axon/libaxon_pjrt.so")
        mod._hook = hook
        mod.set_axon_ntff_profile_hook = lambda h: setattr(mod, "_hook", h)
        mod.get_axon_ntff_profile_hook = lambda: mod._hook
        sys.modules["antenv.axon_hooks"] = mod
        antenv.axon_hooks = mod
    except Exception:
        pass


def _run(inputs, trace=False, **kwargs):
    if trace:
        _install_ntff_shim()
    nc = _get_nc()
    maps = _in_maps(**{k: np.asarray(v) for k, v in inputs.items()})
    return bass_utils.run_bass_kernel_spmd(
        nc, maps, core_ids=list(range(N_CORES)), trace=trace, **kwargs
    )


def _assemble(results):
    cw = np.concatenate(
        [results[c]["cw"].reshape(S_LOC, E, C) for c in range(N_CORES)], axis=0
    )
    me_sum = np.zeros(E, np.float64)
    ce_sum = np.zeros(E, np.float64)
    for c in range(N_CORES):
        part = results[c]["partials"].reshape(2 * E)
        me_sum += part[:E]
        ce_sum += part[E:]
    l_aux = np.float32(E * np.sum(me_sum * ce_sum) / (S * S))
    return l_aux, cw


def kernel(**inputs):
    res = _run(inputs)
    return _assemble(res.results)
